# revision 38
# baseline (speedup 1.0000x reference)
"""Kernel-target-alignment loss on 8 TRN2 NeuronCores (v5).

Math: Xs = X*sqrt(params); d2_ij = ||Xs_i - Xs_j||^2; K = exp(-d2) (diag == 1);
kta = sum(K*tt^T) / (N*sqrt(sum(K*K)));  return -kta.

Design:
  * Symmetry: 8 diagonal supertiles (weight 1) + 28 strictly-upper (weight 2)
    = 36/64 of the [128,1024] tiles.  Tiles with column supertile ct=c exist
    for row blocks rb in [0, 8(c+1)); each core takes those with
    rb === core (mod 8) -> identical slot sequence on every core (SPMD).
    Per-core variation lives in host-packed inputs (layout/dtype only):
    xb = bf16(X^T), xlpb = bf16(X^T) columns per slot, tp = wgt * t block.
  * A = -d2 via one bf16 matmul, K=128 zero-padded: lhsT = [bf16(2p*xb-pack);
    ones; 0], rhs = [xb; srow; 0] where srow_j = bf16(-sum_d p_d xb_dj^2).
    srow comes from PE column-reduces of z = xb^2 with weights -bf16(p),
    three reduces per PSUM tile (rows 0/32/64), one 65-lane copy to SBUF,
    then tiny row DMAs into XSR row 64.  ACT exp bias b_i = 2*s_i -
    f32(bf16(s_i)) from an identically computed slot-packed reduce bounced
    through DRAM ([36,128] readback + PE transpose), so A_ii ~ 0 +- 0.03 and
    exp(A_ii) == 1 +- 3% (s1 impact ~6e-4).
  * K=128 note: bf16 matmuls with 128-col weights (FWL) and K<128 never
    un-throttle the PE HAM clock gate (stuck 1.2 GHz); zero-padding K to 128
    plus a small warmup burst keeps the PE at 2.4 GHz.
  * E = exp(A + b) bf16 on ACT.  s1: DVE scalar_tensor_tensor E*E with accum
    per slot.  s2: two M=1 PE matmuls tp_slot^T @ E into PSUM row 32*(ct%3)
    of strip ct//3 (accumulated across the ct group), drained per strip.
  * Host: s1 = sum_slots wgt * sum(s1o[:, slot]);
    s2 = sum_ct dot(wo[ct], t[ct*1024:+1024]); return -s2 / (N*sqrt(s1)).
"""

import numpy as np

import concourse.bass as bass
import concourse.bacc as bacc
import concourse.tile as tile
import concourse.mybir as mybir
from concourse.bass_utils import run_bass_kernel_spmd

N = 8192
D = 64
NCORES = 8
CW = 1024
NST = 8
NTILES = 36
PK = NTILES * 128          # 4608

F32 = mybir.dt.float32
BF16 = mybir.dt.bfloat16

SLOT_CT = [c for c in range(NST) for _ in range(c + 1)]
assert len(SLOT_CT) == NTILES


def slot_rbs(core):
    return [8 * j + core for c in range(NST) for j in range(c + 1)]


def slot_weights(core):
    w = []
    for c in range(NST):
        for j in range(c + 1):
            rb = 8 * j + core
            w.append(1.0 if 8 * c <= rb < 8 * (c + 1) else 2.0)
    return w


def _ap(tensor, ap, offset=0):
    return bass.AP(tensor=tensor, offset=offset, ap=ap)


def build_kernel():
    nc = bacc.Bacc("TRN2", target_bir_lowering=False)

    xb_d = nc.dram_tensor("xb", [D, N], BF16, kind="ExternalInput")
    xlpb_d = nc.dram_tensor("xlpb", [D, PK], BF16, kind="ExternalInput")
    tp_d = nc.dram_tensor("tp", [128, NTILES], F32, kind="ExternalInput")
    params_d = nc.dram_tensor("params", [D], F32, kind="ExternalInput")
    ident_d = nc.dram_tensor("ident36", [36, 36], F32, kind="ExternalInput")
    zeros_d = nc.dram_tensor("zeros64", [64, N], BF16, kind="ExternalInput")
    ones_d = nc.dram_tensor("ones1", [1, PK], BF16, kind="ExternalInput")
    spackf_d = nc.dram_tensor("spackf_scratch", [PK], F32)
    s1o_d = nc.dram_tensor("s1o", [128, NTILES], F32, kind="ExternalOutput")
    wo_d = nc.dram_tensor("wo", [NST, CW], F32, kind="ExternalOutput")

    with tile.TileContext(nc) as tc:
        with (
            tc.tile_pool(name="const", bufs=1) as cpool,
            tc.tile_pool(name="etile", bufs=8) as epool,
            tc.tile_pool(name="scratch", bufs=4) as spool,
            tc.tile_pool(name="mmpsum", bufs=2, space="PSUM") as mpool,
            tc.tile_pool(name="wq", bufs=2, space="PSUM") as wpool,
        ):
            qpool = wpool
            # ---- persistent SBUF tensors -------------------------------------
            xlpbsb = cpool.tile([D, PK], BF16, tag="xlpbsb")
            XSR = cpool.tile([128, N], BF16, tag="XSR")      # [xb; srow; 0s]
            XSLp = cpool.tile([128, PK], BF16, tag="XSLp")   # [2p*xb-pack; 1s; 0s]
            zz = cpool.tile([D, N], BF16, tag="zz")
            zp = cpool.tile([D, PK], BF16, tag="zp")
            psb = cpool.tile([D, 1], F32, tag="psb")
            rp2 = cpool.tile([D, 1], F32, tag="rp2")
            negp = cpool.tile([D, 1], BF16, tag="negp")
            qsbg = [cpool.tile([65, 512], BF16, tag=f"qsbg{i}", name=f"qsbg{i}")
                    for i in range(6)]
            qsbp = [cpool.tile([65, 512], F32, tag=f"qsbp{i}", name=f"qsbp{i}")
                    for i in range(3)]
            sp36 = cpool.tile([36, 128], F32, tag="sp36")
            ident = cpool.tile([36, 36], F32, tag="ident")
            spackf = cpool.tile([128, NTILES], F32, tag="spackf")
            spackb = cpool.tile([128, NTILES], BF16, tag="spackb")
            spackbf = cpool.tile([128, NTILES], F32, tag="spackbf")
            biasp = cpool.tile([128, NTILES], F32, tag="biasp")
            biasp2 = cpool.tile([128, NTILES], F32, tag="biasp2")
            tpackf = cpool.tile([128, NTILES], F32, tag="tpackf")
            tpackb = cpool.tile([128, NTILES], BF16, tag="tpackb")
            s1acc = cpool.tile([128, NTILES], F32, tag="s1acc")
            wsb = cpool.tile([65, 3 * CW], F32, tag="wsb")
            wcol = cpool.tile([128, 1], BF16, tag="wcol")
            wrhs = cpool.tile([128, 512], BF16, tag="wrhs")

            # ---- input DMAs (sync queue issues fast and spreads across the
            # 16 DMA engines; ordered by when consumers need the data) --------
            nc.sync.dma_start(out=psb[:, :], in_=_ap(params_d, [[1, D], [0, 1]]))
            for s in range(3):
                sl = slice(s * 1536, (s + 1) * 1536)
                nc.sync.dma_start(out=xlpbsb[:, sl], in_=xlpb_d[:, sl])
            for s in range(8):
                sl = slice(s * 1024, (s + 1) * 1024)
                nc.sync.dma_start(out=XSR[0:D, sl], in_=xb_d[:, sl])
            nc.sync.dma_start(out=tpackf[:, :], in_=tp_d[:, :])
            nc.sync.dma_start(out=ident[:, :], in_=ident_d[:, :])
            nc.sync.dma_start(out=XSLp[D : 128, :], in_=zeros_d[0:D, 0:PK])
            nc.sync.dma_start(out=XSLp[D : D + 1, :], in_=ones_d[:, :])
            for s in range(4):
                sl = slice(s * 2048, (s + 1) * 2048)
                nc.sync.dma_start(out=XSR[D : 128, sl], in_=zeros_d[0:D, sl])

            # ---- PE warmup (K=128 M=1 counts as HAM-busy) --------------------
            nc.gpsimd.memset(wcol[:, :], 0.5)
            nc.gpsimd.memset(wrhs[:, :], 0.5)

            def warm(n):
                for _ in range(n):
                    q = qpool.tile([1, 512], F32, tag="qps", name="wq")
                    nc.tensor.matmul(q[0:1, :], wcol[:, :], wrhs[:, :],
                                     start=True, stop=True)

            warm(14)

            def gsq(s):
                sl = slice(s * 1024, (s + 1) * 1024)
                if s % 2 == 0:
                    nc.scalar.activation(out=zz[:, sl], in_=XSR[0:D, sl],
                                         func=mybir.ActivationFunctionType.Square)
                else:
                    nc.vector.tensor_mul(zz[:, sl], XSR[0:D, sl], XSR[0:D, sl])

            # ---- small setup -------------------------------------------------
            nc.vector.tensor_scalar_mul(rp2[:, :], psb[:, :], 2.0)
            nc.vector.tensor_scalar_mul(negp[:, :], psb[:, :], -1.0)
            nc.vector.tensor_copy(out=tpackb[:, :], in_=tpackf[:, :])

            def gsq(s):
                sl = slice(s * 1024, (s + 1) * 1024)
                if s % 2 == 0:
                    nc.scalar.activation(out=zz[:, sl], in_=XSR[0:D, sl],
                                         func=mybir.ActivationFunctionType.Square)
                else:
                    nc.vector.tensor_mul(zz[:, sl], XSR[0:D, sl], XSR[0:D, sl])

            def lcast(s):
                sl = slice(s * 1024, min((s + 1) * 1024, PK))
                nc.vector.tensor_scalar_mul(XSLp[0:D, sl], xlpbsb[:, sl], rp2[:, :])

            # ---- packed side first (gates the exp bias); lhs casts for late
            # slots and late zz squares are deferred into the main loop ------
            for s in range(5):
                sl = slice(s * 1024, min((s + 1) * 1024, PK))
                if s % 2 == 0:
                    nc.scalar.activation(out=zp[:, sl], in_=xlpbsb[:, sl],
                                         func=mybir.ActivationFunctionType.Square)
                else:
                    nc.vector.tensor_mul(zp[:, sl], xlpbsb[:, sl], xlpbsb[:, sl])
            for s in range(4):
                gsq(s)
            for s in range(5):
                lcast(s)
            q3 = None
            for r in range(9):
                ssl = slice(r * 512, (r + 1) * 512)
                row = 32 * (r % 3)
                if r % 3 == 0:
                    q3 = qpool.tile([65, 512], F32, tag="qps", name=f"q3p{r}")
                nc.tensor.matmul(q3[row : row + 1, :], negp[:, :], zp[:, ssl],
                                 start=True, stop=True)
                if r % 3 == 2:
                    k = r // 3
                    if k % 2 == 0:
                        nc.scalar.copy(out=qsbp[k][:, :], in_=q3[:, :])
                    else:
                        nc.vector.tensor_copy(out=qsbp[k][:, :], in_=q3[:, :])
                    warm(1)
            for k in range(3):
                nc.gpsimd.dma_start(
                    out=_ap(spackf_d, [[512, 3], [1, 512]], offset=k * 1536),
                    in_=qsbp[k][0:65:32, :],
                )

            # bias chain: contiguous readback as [36,128] + PE transpose
            nc.gpsimd.dma_start(out=sp36[:, :], in_=_ap(spackf_d, [[128, 36], [1, 128]]))
            qt = qpool.tile([128, 36], F32, tag="qps", name="qt")
            nc.tensor.transpose(qt[:, :], sp36[:, :], ident[:, :])
            nc.vector.tensor_copy(out=spackf[:, :], in_=qt[:, :])
            nc.vector.tensor_copy(out=spackb[:, :], in_=spackf[:, :])
            nc.vector.tensor_copy(out=spackbf[:, :], in_=spackb[:, :])
            nc.vector.scalar_tensor_tensor(
                out=biasp[:, :], in0=spackf[:, :], scalar=2.0, in1=spackbf[:, :],
                op0=mybir.AluOpType.mult, op1=mybir.AluOpType.subtract,
            )
            nc.vector.tensor_scalar_mul(biasp2[:, :], biasp[:, :], 2.0)
            warm(2)

            # ---- global srow triples: triple k covers XSR row-64 columns
            # [1536k, 1536(k+1)); only triples 0-1 (ct 0-2) are needed before
            # the main loop, the rest are emitted interleaved into it --------
            def gtriple(k):
                q3 = qpool.tile([65, 512], F32, tag="qps", name=f"q3g{k}")
                nr = 3 if k < 5 else 1
                for j in range(nr):
                    r = 3 * k + j
                    ssl = slice(r * 512, (r + 1) * 512)
                    nc.tensor.matmul(q3[32 * j : 32 * j + 1, :], negp[:, :],
                                     zz[:, ssl], start=True, stop=True)
                npp = 32 * (nr - 1) + 1
                if k % 2 == 0:
                    nc.vector.tensor_copy(out=qsbg[k][0:npp, :], in_=q3[0:npp, :])
                else:
                    nc.scalar.copy(out=qsbg[k][0:npp, :], in_=q3[0:npp, :])
                nc.sync.dma_start(
                    out=XSR[D : D + 1, slice(k * 1536, k * 1536 + nr * 512)],
                    in_=qsbg[k][0 : 32 * (nr - 1) + 1 : 32, :],
                )

            gtriple(0)
            warm(2)

            # ---- main loop (software pipelined) ------------------------------
            wtiles = {}
            mms = {}

            def stage_a(i):
                ct = SLOT_CT[i]
                first = i == 0 or SLOT_CT[i - 1] != ct
                k = ct // 3
                if first and ct % 3 == 0:
                    nparts = 65 if k < 2 else 34
                    wtiles[k] = wpool.tile(
                        [nparts, CW], F32, tag="wps", name=f"wt{k}", bufs=1
                    )
                lhsT = XSLp[0:128, i * 128 : (i + 1) * 128]
                mm = mpool.tile([128, CW], F32, tag="mm", name="mm")
                for j in range(CW // 512):
                    sl = slice(ct * CW + j * 512, ct * CW + (j + 1) * 512)
                    nc.tensor.matmul(
                        mm[:, j * 512 : (j + 1) * 512], lhsT, XSR[0:128, sl],
                        start=True, stop=True,
                    )
                mms[i] = mm

            ACT2 = {2, 8, 12}  # s1 via exp(2A+2b) on ACT for these slots

            def stage_b(i):
                ct = SLOT_CT[i]
                first = i == 0 or SLOT_CT[i - 1] != ct
                last = i == NTILES - 1 or SLOT_CT[i + 1] != ct
                k, row = ct // 3, 32 * (ct % 3)
                wt = wtiles[k]
                E = epool.tile([128, CW], BF16, tag="E", name="E")
                mm = mms.pop(i)
                nc.scalar.activation(
                    out=E[:, :], in_=mm[:, :],
                    func=mybir.ActivationFunctionType.Exp,
                    bias=biasp[:, i : i + 1], scale=1.0,
                )
                sc1 = spool.tile([128, CW], BF16, tag="sc1", name="sc1", padded_shape=[128, CW + 32])
                if i in ACT2:
                    nc.scalar.activation(
                        out=sc1[:, :], in_=mm[:, :],
                        func=mybir.ActivationFunctionType.Exp,
                        bias=biasp2[:, i : i + 1], scale=2.0,
                        accum_out=s1acc[:, i : i + 1],
                    )
                else:
                    nc.vector.scalar_tensor_tensor(
                        out=sc1[:, :], in0=E[:, :], scalar=1.0, in1=E[:, :],
                        op0=mybir.AluOpType.mult, op1=mybir.AluOpType.mult,
                        accum_out=s1acc[:, i : i + 1],
                    )
                for j in range(CW // 512):
                    nc.tensor.matmul(
                        wt[row : row + 1, j * 512 : (j + 1) * 512],
                        tpackb[:, i : i + 1],
                        E[:, j * 512 : (j + 1) * 512],
                        start=first, stop=last,
                    )
                if last and (ct % 3 == 2 or ct == NST - 1):
                    npp = 65 if k < 2 else 34
                    if k == 1:
                        nc.vector.tensor_copy(
                            out=wsb[0:npp, k * CW : (k + 1) * CW], in_=wt[:, :])
                    else:
                        nc.scalar.copy(
                            out=wsb[0:npp, k * CW : (k + 1) * CW], in_=wt[:, :])
                    for c2 in range(3 * k, min(3 * k + 3, NST)):
                        rr = 32 * (c2 % 3)
                        nc.sync.dma_start(
                            out=wo_d[c2 : c2 + 1, :],
                            in_=wsb[rr : rr + 1, k * CW : (k + 1) * CW],
                        )


            DEFER = {0: [lambda: gsq(4), lambda: gsq(5), lambda: gtriple(2)],
                     2: [lambda: gsq(6), lambda: gtriple(3)],
                     8: [lambda: gsq(7), lambda: gtriple(4)],
                     12: [lambda: gtriple(5)]}

            # A(0) only needs srow columns [0,1536) (triple 0): emit it before
            # triple 1 so the PE reaches the main loop sooner
            stage_a(0)
            gtriple(1)
            warm(2)
            for i in range(NTILES):
                if i + 1 < NTILES:
                    stage_a(i + 1)
                stage_b(i)
                for fn in DEFER.get(i, []):
                    fn()
                if i == 27:
                    nc.sync.dma_start(out=s1o_d[:, 0:28], in_=s1acc[:, 0:28])

            nc.sync.dma_start(out=s1o_d[:, 28:NTILES], in_=s1acc[:, 28:NTILES])

    nc.compile()
    return nc


_NC_CACHE = None


def make_in_maps(X, target, params):
    import ml_dtypes

    X = np.ascontiguousarray(X, dtype=np.float32)
    target = np.ascontiguousarray(target, dtype=np.float32)
    params = np.ascontiguousarray(params, dtype=np.float32)
    xb = np.ascontiguousarray(X.T).astype(ml_dtypes.bfloat16)
    ident = np.eye(36, dtype=np.float32)
    zeros = np.zeros((64, N), dtype=ml_dtypes.bfloat16)
    ones = np.ones((1, PK), dtype=ml_dtypes.bfloat16)
    maps = []
    for c in range(NCORES):
        rbs = slot_rbs(c)
        wgt = slot_weights(c)
        xlpb = np.concatenate(
            [xb[:, rb * 128 : (rb + 1) * 128] for rb in rbs], axis=1
        )
        tp = np.stack(
            [w * target[rb * 128 : (rb + 1) * 128] for rb, w in zip(rbs, wgt)], axis=1
        )
        maps.append({
            "xb": xb,
            "xlpb": np.ascontiguousarray(xlpb),
            "tp": np.ascontiguousarray(tp.astype(np.float32)),
            "params": params,
            "ident36": ident,
            "zeros64": zeros,
            "ones1": ones,
        })
    return maps


def kernel(X, target, params):
    global _NC_CACHE
    X = np.ascontiguousarray(X, dtype=np.float32)
    target = np.ascontiguousarray(target, dtype=np.float32)
    params = np.ascontiguousarray(params, dtype=np.float32)

    in_maps = make_in_maps(X, target, params)

    if _NC_CACHE is None:
        _NC_CACHE = build_kernel()
    res = run_bass_kernel_spmd(_NC_CACHE, in_maps, core_ids=list(range(NCORES)))

    s1 = 0.0
    s2 = 0.0
    t64 = target.astype(np.float64)
    for c in range(NCORES):
        wgt = slot_weights(c)
        s1o = res.results[c]["s1o"].astype(np.float64)
        wo = res.results[c]["wo"].astype(np.float64)
        for i in range(NTILES):
            s1 += wgt[i] * float(s1o[:, i].sum())
        for ct in range(NST):
            s2 += float(np.dot(wo[ct], t64[ct * CW : (ct + 1) * CW]))

    val = -s2 / (N * np.sqrt(s1))
    return np.array(val, dtype=np.float32)
